# revision 1
# baseline (speedup 1.0000x reference)
"""Trainium2 Bass kernel for nn_Attention2D_ROPE (B=8, N=1024, C=1024, 16 heads).

Strategy: pure data parallelism — batch 8 sharded one-per-core across the 8
NeuronCores; no collectives. Per core: QKV GEMM -> 2D RoPE -> attention
(S^T layout so softmax sums come out of the AV matmul via an appended ones
column) -> out projection. All matmuls run in fp16 with fp32 PSUM
accumulation (fp16 mantissa is plenty for this data range and is ~4x faster
than fp32 on the PE).

Self-contained: hardcodes all shapes; host-side numpy does the sharding,
layout tiling, weight permutation and fp16 casts.
"""
import numpy as np
from contextlib import ExitStack

B, GH, GW, NH, C = 8, 32, 32, 16, 1024
HD = C // NH          # 64
N = GH * GW           # 1024
KO = C // 128         # 8 contraction chunks
NCORES = 8

_CACHE = {}


# ---------------------------------------------------------------- host prep

def _rope_tables():
    # identical to the reference's _rope_cos_sin
    theta = 1.0 / (10000.0 ** (np.arange(0, HD // 2, 2, dtype=np.float32) / (HD // 2)))
    ang_h = np.arange(GH, dtype=np.float32)[:, None] * theta[None, :]
    ang_w = np.arange(GW, dtype=np.float32)[:, None] * theta[None, :]
    ang = np.zeros((GH, GW, HD // 2), dtype=np.float32)
    ang[..., 0::2] = ang_h[:, None, :]
    ang[..., 1::2] = ang_w[None, :, :]
    ang = ang.reshape(N, HD // 2)  # (N, 32)
    return np.cos(ang), np.sin(ang)


def _qk_row_perm():
    """Order of wqkv rows for the qk GEMM output chunks.

    Chunk (g, t, ri) holds, for head group g (heads 4g..4g+3), tensor t
    (q=0/k=1), the r (ri=0) or i (ri=1) halves of the RoPE pairs:
    partition p = s*32 + j  ->  original row t*C + (4g+s)*64 + 2j + ri.
    Chunks are emitted g-major so a head group's q AND k finish together.
    """
    rows = []
    for g in range(4):
        for t in range(2):
            for ri in range(2):
                for s in range(4):
                    for j in range(32):
                        rows.append(t * C + (4 * g + s) * 64 + 2 * j + ri)
    return np.array(rows, dtype=np.int64)  # (2048,)


def _prep_shared(wqkv_w, wqkv_b, out_w, out_b):
    f16 = np.float16
    perm = _qk_row_perm()
    wqk = wqkv_w[perm]            # (2048, C)
    # wqkA[c, p, ko, m] = wqk[c*128+m, ko*128+p]
    wqkA = np.ascontiguousarray(
        wqk.reshape(16, 128, KO, 128).transpose(0, 3, 2, 1)
    ).astype(f16)
    bqkA = wqkv_b[perm].reshape(1, 2048).astype(np.float16)
    # wvA[p, ko, j] = wqkv_w[2C + j, ko*128+p]
    wvA = np.ascontiguousarray(
        wqkv_w[2 * C:].reshape(C, KO, 128).transpose(2, 1, 0)
    ).astype(f16)
    vb = wqkv_b[2 * C:].astype(np.float16).reshape(1, C)
    outwA = np.ascontiguousarray(
        out_w.reshape(C, KO, 128).transpose(2, 1, 0)
    ).astype(f16)
    ob = out_b.astype(np.float16).reshape(1, C)
    cos_t, sin_t = _rope_tables()           # (N, 32)
    cosA = np.ascontiguousarray(np.tile(cos_t.T, (4, 1))).astype(f16)  # (128, N)
    sinA = np.ascontiguousarray(np.tile(sin_t.T, (4, 1))).astype(f16)
    return dict(wqkA=wqkA, bqk=bqkA, wvA=wvA, vb=vb, outwA=outwA, ob=ob,
                cosA=cosA, sinA=sinA)


# ------------------------------------------------------------- device build

def _build_module(opts=None):
    import concourse.bass as bass
    import concourse.tile as tile
    from concourse import bacc, mybir

    o = dict(phase1=True, phase2=True, phase3=True, interleave=False,
             gpsum_bufs=2, apsum_bufs=2, qpsum_bufs=2, attn_bufs=3, wqk_bufs=3,
             reps=1,
             debug=False, norm="batched", has_bias=True, fuse_slots=False,
             split_outproj=True, hiprio_attn=False, copy_eng="any",
             rep_bufs=3, stage_bufs=2, pre_bufs=2, rot_bufs=2, oraw_bufs=16,
             norm_batches=2, split_in_dma=True, v_order=0, big_exp=False)
    o.update(opts or {})

    f16, f32 = mybir.dt.float16, mybir.dt.float32
    ts, ds = bass.ts, bass.ds
    Exp = mybir.ActivationFunctionType.Exp
    Ln = mybir.ActivationFunctionType.Ln

    nc = bacc.Bacc("TRN2", target_bir_lowering=False, debug=False)

    xA = nc.dram_tensor("xA", [128, KO, N], f16, kind="ExternalInput")
    wqkA = nc.dram_tensor("wqkA", [16, 128, KO, 128], f16, kind="ExternalInput")
    bqk = nc.dram_tensor("bqk", [1, 2048], f16, kind="ExternalInput")
    wvA = nc.dram_tensor("wvA", [128, KO, C], f16, kind="ExternalInput")
    vb = nc.dram_tensor("vb", [1, C], f16, kind="ExternalInput")
    outwA = nc.dram_tensor("outwA", [128, KO, C], f16, kind="ExternalInput")
    ob = nc.dram_tensor("ob", [1, C], f16, kind="ExternalInput")
    cosA = nc.dram_tensor("cosA", [128, N], f16, kind="ExternalInput")
    sinA = nc.dram_tensor("sinA", [128, N], f16, kind="ExternalInput")
    out = nc.dram_tensor("out", [N, C], f32, kind="ExternalOutput")

    with tile.TileContext(nc) as tc, ExitStack() as ctx:
        const = ctx.enter_context(tc.tile_pool(name="const", bufs=1))
        wqk_pool = ctx.enter_context(tc.tile_pool(name="wqk", bufs=o["wqk_bufs"]))
        pre_pool = ctx.enter_context(tc.tile_pool(name="pre", bufs=o["pre_bufs"]))
        rtmp_pool = ctx.enter_context(tc.tile_pool(name="rtmp", bufs=1))
        rot_pool = ctx.enter_context(tc.tile_pool(name="rot", bufs=o["rot_bufs"]))
        attn_pool = ctx.enter_context(tc.tile_pool(name="attn", bufs=o["attn_bufs"]))
        recip_pool = ctx.enter_context(tc.tile_pool(name="recip", bufs=2))
        rep_pool = ctx.enter_context(tc.tile_pool(name="rep", bufs=o["rep_bufs"]))
        stage_pool = ctx.enter_context(tc.tile_pool(name="stage", bufs=o["stage_bufs"]))
        oraw_pool = ctx.enter_context(tc.tile_pool(name="oraw", bufs=o["oraw_bufs"]))
        rin_pool = ctx.enter_context(tc.tile_pool(name="rin", bufs=2))
        outsb_pool = ctx.enter_context(tc.tile_pool(name="outsb", bufs=2))
        gpsum = ctx.enter_context(
            tc.tile_pool(name="gpsum", bufs=o["gpsum_bufs"], space="PSUM"))
        qpsum = ctx.enter_context(
            tc.tile_pool(name="qpsum", bufs=o["qpsum_bufs"], space="PSUM"))
        apsum = ctx.enter_context(
            tc.tile_pool(name="apsum", bufs=o["apsum_bufs"], space="PSUM"))
        dscr_pool = ctx.enter_context(
            tc.tile_pool(name="dscr", bufs=4, space="DRAM"))

        # ---- resident inputs
        xT = const.tile([128, KO, N], f16)
        wv_sb = const.tile([128, KO, C], f16)
        outw_sb = const.tile([128, KO, C], f16)
        if o["split_in_dma"]:
            for ko in range(KO):
                nc.sync.dma_start(xT[:, ko, :], xA.ap()[:, ko, :])
            for ko in range(KO):
                nc.sync.dma_start(wv_sb[:, ko, :], wvA.ap()[:, ko, :])
            for ko in range(KO):
                nc.sync.dma_start(outw_sb[:, ko, :], outwA.ap()[:, ko, :])
        else:
            nc.sync.dma_start(xT[:], xA.ap())
            nc.sync.dma_start(wv_sb[:], wvA.ap())
            nc.sync.dma_start(outw_sb[:], outwA.ap())
        cos_sb = const.tile([128, N], f16)
        nc.sync.dma_start(cos_sb[:], cosA.ap())
        sin_sb = const.tile([128, N], f16)
        nc.sync.dma_start(sin_sb[:], sinA.ap())
        bqk_sb = const.tile([1, 2048], f16)
        nc.sync.dma_start(bqk_sb[:], bqk.ap())
        vb_sb = const.tile([1, C], f16)
        nc.sync.dma_start(vb_sb[:], vb.ap())
        ob_sb = const.tile([1, C], f16)
        nc.sync.dma_start(ob_sb[:], ob.ap())
        ones_row = const.tile([1, N], f16)
        nc.vector.memset(ones_row[:], 1.0)

        v_aug = const.tile([128, KO, 16 * 65], f16)   # per head: 64 v cols + ones
        ones_cols = v_aug[:].rearrange("p c (h e) -> p c h e", e=65)[:, :, :, 64:65]
        nc.vector.memset(ones_cols, 1.0)
        qT = const.tile([128, NH // 2, N], f16)       # chunk j: heads 2j, 2j+1
        kT = const.tile([128, NH // 2, N], f16)
        oT = const.tile([128, KO, N], f16)

        # ---- phase 1a: v GEMM (token-major) into v_aug
        def emit_v_gemm():
            for mt in range(8):
                for f in range(2):
                    pv = gpsum.tile([128, 512], f32, tag="g")
                    first = True
                    if o["has_bias"]:
                        nc.tensor.matmul(   # bias via ones-row, K=1
                            pv[:], ones_row[0:1, 0:128],
                            vb_sb[0:1, ds(f * 512, 512)], start=True, stop=False)
                        first = False
                    for ko in range(KO):
                        nc.tensor.matmul(
                            pv[:],
                            xT[:, ko, ts(mt, 128)],
                            wv_sb[:, ko, ds(f * 512, 512)],
                            start=first, stop=(ko == KO - 1),
                        )
                        first = False
                    dst = v_aug[:, mt, :].rearrange("p (h e) -> p h e", e=65)[
                        :, ds(f * 8, 8), 0:64]
                    ce = nc.vector if o["copy_eng"] == "vector" else nc.any
                    ce.tensor_copy(
                        out=dst,
                        in_=pv[:].rearrange("p (h d) -> p h d", d=64),
                    )

        # ---- phase 1b: qk GEMM + RoPE for one head group g (4 heads)
        def emit_qk_group(g):
            for t in range(2):  # q, k
                pre = pre_pool.tile([128, 2, N], f16, tag="pre")
                for ri in range(2):
                    m = g * 4 + t * 2 + ri  # wqkA chunk index
                    wt = wqk_pool.tile([128, KO, 128], f16, tag="wqk")
                    nc.sync.dma_start(wt[:], wqkA.ap()[m])
                    for f in range(2):
                        pq = gpsum.tile([128, 512], f32, tag="g")
                        first = True
                        if o["has_bias"]:
                            nc.tensor.matmul(
                                pq[:],
                                bqk_sb[0:1, ds(m * 128, 128)],
                                ones_row[0:1, ds(f * 512, 512)],
                                start=True, stop=False)
                            first = False
                        for ko in range(KO):
                            nc.tensor.matmul(
                                pq[:],
                                wt[:, ko, :],
                                xT[:, ko, ds(f * 512, 512)],
                                start=first, stop=(ko == KO - 1),
                            )
                            first = False
                        ce = nc.vector if o["copy_eng"] == "vector" else nc.any
                        ce.tensor_copy(
                            out=pre[:, ri, ds(f * 512, 512)], in_=pq[:])
                # RoPE: rot_r = xr*cos - xi*sin ; rot_i = xr*sin + xi*cos
                rt = rot_pool.tile([128, 2, N], f16, tag="rot")
                t1 = rtmp_pool.tile([128, N], f16, tag="t1")
                t2 = rtmp_pool.tile([128, N], f16, tag="t2")
                nc.vector.tensor_mul(t1[:], pre[:, 0, :], cos_sb[:])
                nc.vector.tensor_mul(t2[:], pre[:, 1, :], sin_sb[:])
                nc.vector.tensor_sub(rt[:, 0, :], t1[:], t2[:])
                t3 = rtmp_pool.tile([128, N], f16, tag="t1")
                t4 = rtmp_pool.tile([128, N], f16, tag="t2")
                nc.vector.tensor_mul(t3[:], pre[:, 0, :], sin_sb[:])
                nc.vector.tensor_mul(t4[:], pre[:, 1, :], cos_sb[:])
                nc.vector.tensor_add(rt[:, 1, :], t3[:], t4[:])
                # repack to head-major: head 4g+s -> tgt[(h%2)*64.., h//2, :]
                tgt = qT if t == 0 else kT
                for s in range(4):
                    h = 4 * g + s
                    base = (h % 2) * 64
                    for ri in range(2):
                        nc.sync.dma_start(
                            tgt[ds(base + ri * 32, 32), h // 2, :],
                            rt[ds(s * 32, 32), ri, :],
                        )

        # ---- phase 2: attention for one head pair hp
        #      (heads 2hp at partitions 0-63, 2hp+1 at 64-127)
        def dbg_dump(nm, ap, dt_):
            d = nc.dram_tensor(nm, list(ap.shape), dt_, kind="ExternalOutput")
            nc.sync.dma_start(d.ap(), ap)

        def emit_attention(hp, batch_state=None):
            if o["big_exp"]:
                emit_attention_bigexp(hp, batch_state)
                return
            if o["fuse_slots"]:
                emit_attention_fused(hp, batch_state)
                return
            for qh in range(2):
                oA = apsum.tile([65, 512], f32, tag="av")
                oB = apsum.tile([65, 512], f32, tag="av")
                for kc in range(8):
                    ps = qpsum.tile([128, 1024], f32, tag="q")
                    nc.tensor.matmul(
                        ps[:, 0:512],
                        kT[0:64, hp, ts(kc, 128)],
                        qT[0:64, hp, ds(qh * 512, 512)],
                        start=True, stop=True,
                    )
                    nc.tensor.matmul(
                        ps[:, 512:1024],
                        kT[64:128, hp, ts(kc, 128)],
                        qT[64:128, hp, ds(qh * 512, 512)],
                        start=True, stop=True,
                    )
                    at = attn_pool.tile([128, 1024], f16, tag="attn")
                    nc.scalar.activation(at[:], ps[:], Exp, scale=float(HD) ** -0.5)
                    nc.tensor.matmul(
                        oA[:], v_aug[:, kc, ds((2 * hp) * 65, 65)], at[:, 0:512],
                        start=(kc == 0), stop=(kc == 7),
                    )
                    nc.tensor.matmul(
                        oB[:], v_aug[:, kc, ds((2 * hp + 1) * 65, 65)], at[:, 512:1024],
                        start=(kc == 0), stop=(kc == 7),
                    )
                finish_unit(hp, qh, oA, oB, batch_state)

        def emit_attention_bigexp(hp, batch_state):
            # One [128, 2048] QK psum covers (slot x 2 kc chunks); a single
            # exp per psum halves ScalarE per-op overhead. qpsum must be
            # bufs=1 x [128, 2048] for this variant (PSUM bank budget).
            for qh in range(2):
                oA = apsum.tile([65, 512], f32, tag="av")
                oB = apsum.tile([65, 512], f32, tag="av")
                for kc2 in range(4):
                    ps = qpsum.tile([128, 2048], f32, tag="q")
                    for j in range(2):  # kc = 2*kc2 + j
                        kc = 2 * kc2 + j
                        nc.tensor.matmul(
                            ps[:, ds(j * 1024, 512)],
                            kT[0:64, hp, ts(kc, 128)],
                            qT[0:64, hp, ds(qh * 512, 512)],
                            start=True, stop=True,
                        )
                        nc.tensor.matmul(
                            ps[:, ds(j * 1024 + 512, 512)],
                            kT[64:128, hp, ts(kc, 128)],
                            qT[64:128, hp, ds(qh * 512, 512)],
                            start=True, stop=True,
                        )
                    at = attn_pool.tile([128, 2048], f16, tag="attn")
                    nc.scalar.activation(at[:], ps[:], Exp, scale=float(HD) ** -0.5)
                    for j in range(2):
                        kc = 2 * kc2 + j
                        nc.tensor.matmul(
                            oA[:], v_aug[:, kc, ds((2 * hp) * 65, 65)],
                            at[:, ds(j * 1024, 512)],
                            start=(kc == 0), stop=(kc == 7),
                        )
                        nc.tensor.matmul(
                            oB[:], v_aug[:, kc, ds((2 * hp + 1) * 65, 65)],
                            at[:, ds(j * 1024 + 512, 512)],
                            start=(kc == 0), stop=(kc == 7),
                        )
                finish_unit(hp, qh, oA, oB, batch_state)

        def emit_attention_fused(hp, batch_state):
            # All 4 AV psums for the pair live at once; each kT slice and each
            # v_aug slice is the stationary operand for 2 consecutive matmuls
            # (both q-halves), halving LDWEIGHTS traffic. Exp covers one head
            # (both q-halves) per op.
            oo = [[None, None], [None, None]]   # oo[slot][qh]
            for slot in range(2):
                for qh in range(2):
                    avt = apsum.tile([65, 512], f32, tag="av", name=f"av_{slot}_{qh}")
                    oo[slot][qh] = avt
            for kc in range(8):
                ats = []
                for slot in range(2):
                    ps = gpsum.tile([128, 1024], f32, tag="g")
                    for qh in range(2):
                        nc.tensor.matmul(
                            ps[:, ds(qh * 512, 512)],
                            kT[ds(slot * 64, 64), hp, ts(kc, 128)],
                            qT[ds(slot * 64, 64), hp, ds(qh * 512, 512)],
                            start=True, stop=True,
                        )
                    at = attn_pool.tile([128, 1024], f16, tag="attn")
                    nc.scalar.activation(at[:], ps[:], Exp, scale=float(HD) ** -0.5)
                    ats.append(at)
                for slot in range(2):
                    h = 2 * hp + slot
                    for qh in range(2):
                        nc.tensor.matmul(
                            oo[slot][qh][:],
                            v_aug[:, kc, ds(h * 65, 65)],
                            ats[slot][:, ds(qh * 512, 512)],
                            start=(kc == 0), stop=(kc == 7),
                        )
            for qh in range(2):
                finish_unit(hp, qh, oo[0][qh], oo[1][qh], batch_state)

        def finish_unit(hp, qh, oA, oB, batch_state):
            if True:
                if o["debug"] and hp == 0 and qh == 0:
                    oc = const.tile([65, 1024], f32)
                    nc.vector.tensor_copy(out=oc[:, 0:512], in_=oA[:])
                    nc.vector.tensor_copy(out=oc[:, 512:1024], in_=oB[:])
                    dbg_dump("dbg_oAB00", oc[:], f32)
                # escape o (unnormalized) from PSUM; gather sums (row 64) into
                # rin for a batched reciprocal later. (Per-unit reciprocal on
                # ScalarE would thrash ACT table sets; custom DVE recip ops are
                # broken on this runtime; DVE divide is FD-bound, so batch it.)
                rin, units = batch_state
                u = len(units)
                ss = recip_pool.tile([65, 1024], f32, tag="rc")
                nc.vector.tensor_copy(ss[64:65, 0:512], oA[64:65, :])
                nc.vector.tensor_copy(ss[64:65, 512:1024], oB[64:65, :])
                nc.sync.dma_start(rin[u:u + 1, :], ss[64:65, :])
                orA = oraw_pool.tile([64, 512], f16, tag="or")
                nc.vector.tensor_copy(orA[:], oA[0:64, :])
                orB = oraw_pool.tile([64, 512], f16, tag="or")
                nc.vector.tensor_copy(orB[:], oB[0:64, :])
                units.append((hp, qh, u, orA, orB))

        def emit_norm_batch(rin, units):
            rrec = recip_pool.tile([8, 1024], f32, tag="rc")
            dscr = dscr_pool.tile([8, 1024], f32, tag="dscr")
            for h2 in range(2):   # split so qh=0 broadcasts start sooner
                nc.vector.reciprocal(
                    rrec[:, ds(h2 * 512, 512)], rin[:, ds(h2 * 512, 512)])
                nc.sync.dma_start(
                    dscr[:, ds(h2 * 512, 512)], rrec[:, ds(h2 * 512, 512)])
            for (hp, qh, u, orA, orB) in units:
                repA = rep_pool.tile([64, 512], f32, tag="rep")
                nc.sync.dma_start(
                    repA[:], dscr[u:u + 1, 0:512].to_broadcast((64, 512)))
                repB = rep_pool.tile([64, 512], f32, tag="rep")
                nc.sync.dma_start(
                    repB[:], dscr[u:u + 1, 512:1024].to_broadcast((64, 512)))
                nc.vector.tensor_mul(
                    oT[0:64, hp, ds(qh * 512, 512)], orA[:], repA[:])
                stg = stage_pool.tile([64, 512], f16, tag="stg")
                nc.vector.tensor_mul(stg[:], orB[:], repB[:])
                nc.sync.dma_start(oT[ds(64, 64), hp, ds(qh * 512, 512)], stg[:])

        # ---- phase 3: out projection (optionally split over K feature
        #      chunks, accumulating the second half into DRAM)
        def emit_out_proj(kos=range(KO), eco=True, accum=False):
            kos = list(kos)
            for mt in range(8):
                osb = outsb_pool.tile([128, C], f32, tag="osb")
                for f in range(2):
                    po = gpsum.tile([128, 512], f32, tag="g")
                    first = True
                    if o["has_bias"] and eco:
                        nc.tensor.matmul(
                            po[:], ones_row[0:1, 0:128],
                            ob_sb[0:1, ds(f * 512, 512)], start=True, stop=False)
                        first = False
                    for ko in kos:
                        nc.tensor.matmul(
                            po[:],
                            oT[:, ko, ts(mt, 128)],
                            outw_sb[:, ko, ds(f * 512, 512)],
                            start=first, stop=(ko == kos[-1]),
                        )
                        first = False
                    ce = nc.vector if o["copy_eng"] == "vector" else nc.any
                    ce.tensor_copy(out=osb[:, ds(f * 512, 512)], in_=po[:])
                dst = out.ap().rearrange("(mt p) j -> p mt j", p=128)[:, mt, :]
                if accum:
                    nc.gpsimd.dma_start(dst, osb[:], accum_op=mybir.AluOpType.add)
                else:
                    nc.sync.dma_start(dst, osb[:])

        for _rep in range(o["reps"]):
            if o["phase1"]:
                emit_v_gemm()
            for g in range(4 if o["phase1"] else 0):
                emit_qk_group(g)
            if o["phase2"]:
                for bt in range(2):   # two recip batches of 4 head pairs
                    rin = rin_pool.tile([8, 1024], f32, tag="rin")
                    units = []
                    for hp in range(4 * bt, 4 * bt + 4):
                        emit_attention(hp, (rin, units))
                    emit_norm_batch(rin, units)
            if o["phase3"]:
                emit_out_proj()

        if o["debug"]:
            for nm, tile_ in (("dbg_qT", qT), ("dbg_kT", kT), ("dbg_oT", oT),
                              ("dbg_vaug", v_aug)):
                shp = list(tile_.shape)
                d = nc.dram_tensor(nm, shp, f16, kind="ExternalOutput")
                nc.sync.dma_start(d.ap(), tile_[:])

    nc.compile()
    return nc


# ---------------------------------------------------------------- execution

class _SpmdRunner:
    """Keeps one jitted shard_map callable over the 8 axon cores."""

    def __init__(self, nc, n_cores=NCORES):
        import jax
        import numpy as np
        from jax.sharding import Mesh, PartitionSpec, NamedSharding
        from jax.experimental.shard_map import shard_map
        import concourse.mybir as mybir
        from concourse.bass2jax import (
            _bass_exec_p, install_neuronx_cc_hook, partition_id_tensor)

        install_neuronx_cc_hook()
        self.jax = jax
        self.nc = nc
        self.n_cores = n_cores
        partition_name = (
            nc.partition_id_tensor.name if nc.partition_id_tensor else None)

        in_names, out_names, out_avals, zero_outs = [], [], [], []
        for alloc in nc.m.functions[0].allocations:
            if not isinstance(alloc, mybir.MemoryLocationSet):
                continue
            name = alloc.memorylocations[0].name
            if alloc.kind == "ExternalInput":
                if name != partition_name:
                    in_names.append(name)
            elif alloc.kind == "ExternalOutput":
                out_names.append(name)
                shape = tuple(alloc.tensor_shape)
                dtype = mybir.dt.np(alloc.dtype)
                out_avals.append(jax.core.ShapedArray(shape, dtype))
                zero_outs.append(np.zeros(shape, dtype))
        self.in_names, self.out_names = in_names, out_names
        self.out_avals, self.zero_outs = out_avals, zero_outs
        n_params, n_outs = len(in_names), len(out_avals)
        all_names = in_names + out_names
        if partition_name is not None:
            all_names = all_names + [partition_name]

        def _body(*args):
            operands = list(args)
            if partition_name is not None:
                operands.append(partition_id_tensor())
            return tuple(_bass_exec_p.bind(
                *operands,
                out_avals=tuple(out_avals),
                in_names=tuple(all_names),
                out_names=tuple(out_names),
                lowering_input_output_aliases=(),
                sim_require_finite=True,
                sim_require_nnan=True,
                nc=nc,
            ))

        devices = jax.devices()[:n_cores]
        mesh = Mesh(np.asarray(devices), ("core",))
        self.sharding = NamedSharding(mesh, PartitionSpec("core"))
        in_specs = (PartitionSpec("core"),) * (n_params + n_outs)
        out_specs = (PartitionSpec("core"),) * n_outs
        self.fn = jax.jit(
            shard_map(_body, mesh=mesh, in_specs=in_specs,
                      out_specs=out_specs, check_rep=False),
            donate_argnums=tuple(range(n_params, n_params + n_outs)),
            keep_unused=True,
        )

    def stage_inputs(self, in_maps):
        import numpy as np
        concat = [
            np.concatenate(
                [np.asarray(in_maps[c][n]) for c in range(self.n_cores)], axis=0)
            for n in self.in_names
        ]
        self.dev_in = [self.jax.device_put(x, self.sharding) for x in concat]

    def stage_zeros(self):
        import numpy as np
        return [
            self.jax.device_put(
                np.zeros((self.n_cores * z.shape[0], *z.shape[1:]), z.dtype),
                self.sharding)
            for z in self.zero_outs
        ]

    def run(self, zeros=None):
        if zeros is None:
            zeros = self.stage_zeros()
        outs = self.fn(*self.dev_in, *zeros)
        self.jax.block_until_ready(outs)
        return outs

    def results(self, out_arrs):
        import numpy as np
        return [
            {n: np.asarray(out_arrs[i]).reshape(
                self.n_cores, *self.out_avals[i].shape)[c]
             for i, n in enumerate(self.out_names)}
            for c in range(self.n_cores)
        ]


def _get_runner(has_bias):
    key = ("runner", has_bias)
    if key not in _CACHE:
        nc = _build_module({"has_bias": has_bias})
        _CACHE[("nc", has_bias)] = nc
        _CACHE["nc"] = nc
        _CACHE[key] = _SpmdRunner(nc)
        _CACHE["runner"] = _CACHE[key]
    return _CACHE[key]


def _make_in_maps(x, wqkv_w, wqkv_b, out_w, out_b):
    shared = _prep_shared(
        np.asarray(wqkv_w, dtype=np.float32),
        np.asarray(wqkv_b, dtype=np.float32),
        np.asarray(out_w, dtype=np.float32),
        np.asarray(out_b, dtype=np.float32),
    )
    x = np.asarray(x, dtype=np.float32)
    in_maps = []
    for b in range(NCORES):
        # xA[p, ko, n] = x[b, n, ko*128+p]
        xb = np.ascontiguousarray(
            x[b].T.reshape(KO, 128, N).transpose(1, 0, 2)).astype(np.float16)
        m = dict(shared)
        m["xA"] = xb
        in_maps.append(m)
    return in_maps


def kernel(x, wqkv_w, wqkv_b, out_w, out_b):
    has_bias = bool(np.any(np.asarray(wqkv_b)) or np.any(np.asarray(out_b)))
    runner = _get_runner(has_bias)
    in_maps = _make_in_maps(x, wqkv_w, wqkv_b, out_w, out_b)
    runner.stage_inputs(in_maps)
    outs = runner.run()
    res = runner.results(outs)
    full = np.stack([res[c]["out"] for c in range(NCORES)], axis=0)
    return (full.astype(np.float32),)

